# revision 12
# baseline (speedup 1.0000x reference)
"""GCN (2-layer, eval-mode BN, ReLU) on 8 Trainium2 NeuronCores.

Strategy: shard nodes across 8 cores (12500 each). Per layer:
  1. local matmul H = X @ (W * bn_scale), scaled per-node by dinv (bf16)
  2. AllGather bf16 node features -> full table in each core's DRAM
  3. per 128-node dst tile: dma_gather (custom Q7 ucode; int16 indices, so
     the table is addressed in 4 source-range rounds) pulls all incident
     edge messages into SBUF; gathers are issued prepare_only on 4 SWDGE
     queues so transfers overlap. The one-hot scatter matrices
     S[e,d] = (dloc[e]==d) are precomputed on host (pure 0/1; both norm
     factors are folded elsewhere), streamed from DRAM, and contracted
     against the gathered messages on the PE (the segment-sum).
  4. epilogue + ReLU; layer-1 result kept transposed in SBUF for layer 2.

All graph preprocessing (sorting edges by (dst tile, src range), padding to
128-edge blocks with a cross-core-uniform schedule, BN constant folding,
one-hot S blocks) happens on host.
"""

import math

import numpy as np
import ml_dtypes

import concourse.bass as bass
import concourse.mybir as mybir
import concourse.tile as tile
from concourse import bacc, library_config
from concourse.bass_utils import run_bass_kernel_spmd
from concourse.bass_interp import get_hw_module
from concourse.masks import make_identity

BF16 = ml_dtypes.bfloat16

N = 100000
E = 3200000
DIN = 256
HID = 128
C = 8
NSH = N // C            # 12500 nodes per core
T = math.ceil(NSH / 128)  # 98 dst tiles per core
NSHP = T * 128          # 12544 padded nodes per core
AGROWS = C * NSHP       # 100352 rows in the gathered table
NQ = 4                  # source-range rounds (int16 gather indices)
GRP = 2                 # dst tiles merged per gather call
BN_EPS = 1e-5
NSWQ = 4                # SWDGE queues for overlapped gathers

_PAD_DLOC = 200.0  # no local dst index matches 200 -> padding slot

USE_PREP = False   # prepare_only + trigger_dma pipelined gathers


def _fold_bn(W, b, gamma, beta, rm, rv):
    s = 1.0 / np.sqrt(np.asarray(rv, np.float64) + BN_EPS)
    A = np.asarray(gamma, np.float64) * s
    Bc = (np.asarray(b, np.float64) - np.asarray(rm, np.float64)) * A + np.asarray(
        beta, np.float64
    )
    Wf = np.asarray(W, np.float64) * A[None, :]
    return Wf.astype(BF16), Bc.astype(np.float32)


def _preprocess(inputs):
    QR = AGROWS // NQ
    assert QR <= 32767

    src = np.asarray(inputs["edge_index"][0], dtype=np.int64)
    dst = np.asarray(inputs["edge_index"][1], dtype=np.int64)
    loops = np.arange(N, dtype=np.int64)
    sa = np.concatenate([src, loops])
    da = np.concatenate([dst, loops])

    deg = np.bincount(da, minlength=N).astype(np.float64)
    dinv = (1.0 / np.sqrt(deg)).astype(np.float32)

    gv = (sa // NSH) * NSHP + (sa % NSH)  # row in the gathered table
    q_arr0 = gv // QR

    c_arr0 = da // NSH
    nloc0 = da - c_arr0 * NSH
    t_arr0 = nloc0 // 128

    NG = T // GRP  # tile groups (one merged gather call per (group, round))
    g_arr0 = t_arr0 // GRP
    tw0 = t_arr0 - g_arr0 * GRP
    key = ((c_arr0 * NG + g_arr0) * NQ + q_arr0) * GRP + tw0
    order = np.argsort(key, kind="stable")
    sa, da, gv, key = sa[order], da[order], gv[order], key[order]
    c_arr = c_arr0[order]
    nloc = nloc0[order]
    dl = (nloc - (nloc // 128) * 128).astype(np.float32)

    NSEC = NG * NQ * GRP
    cnt = np.bincount(key, minlength=C * NSEC)
    starts = np.concatenate([[0], np.cumsum(cnt)[:-1]])
    within = np.arange(len(da)) - starts[key]

    # cross-core-uniform block schedule per (group, round, tile)
    nbq = np.ceil(cnt.reshape(C, NSEC).max(axis=0) / 128).astype(np.int64)
    boffq = np.concatenate([[0], np.cumsum(nbq)[:-1]])
    TBQ = int(nbq.sum())

    tq = key - c_arr * NSEC  # section id per edge
    B_arr = boffq[tq] + within // 128
    p_arr = within % 128

    dloc = np.full((C, 128, TBQ), _PAD_DLOC, np.float32)
    dloc[c_arr, p_arr, B_arr] = dl

    # wrapped int16 index layout for dma_gather: per call (t,q), flat pos i
    # lives at [i%16 + 16k, boffq*8 + i//16] for k in 0..7 (replicated)
    lidx = (gv % QR).astype(np.int16)
    col_g = boffq[tq] * 8 + within // 16
    row_g = within % 16
    gidx16 = np.zeros((C, 128, TBQ * 8), np.int16)
    for k in range(8):
        gidx16[c_arr, row_g + 16 * k, col_g] = lidx

    W1f, B1 = _fold_bn(
        inputs["W1"], inputs["b1"], inputs["gamma1"], inputs["beta1"],
        inputs["rm1"], inputs["rv1"],
    )
    W2f, B2 = _fold_bn(
        inputs["W2"], inputs["b2"], inputs["gamma2"], inputs["beta2"],
        inputs["rm2"], inputs["rv2"],
    )
    B1t = np.ascontiguousarray(np.broadcast_to(B1, (128, HID)))
    B2t = np.ascontiguousarray(np.broadcast_to(B2, (128, HID)))

    x = np.asarray(inputs["x"], np.float32)
    d_iota = np.arange(128, dtype=np.float32)
    in_maps = []
    for c in range(C):
        xs = x[c * NSH : (c + 1) * NSH]  # [NSH, DIN]
        xT = np.zeros((DIN, NSHP), BF16)
        xT[:, :NSH] = xs.T.astype(BF16)
        dvflat = np.zeros(NSHP, np.float32)
        dvflat[:NSH] = dinv[c * NSH : (c + 1) * NSH]
        dinv_t = np.ascontiguousarray(dvflat.reshape(T, 128).T)  # [128, T]
        # host-built one-hot scatter blocks: S[p, b, d] = (dloc[c,p,b] == d)
        S = (dloc[c][:, :, None] == d_iota[None, None, :]).astype(BF16)
        in_maps.append(
            {
                "xT": xT,
                "dinv_t": dinv_t,
                "W1f": W1f,
                "W2f": W2f,
                "B1t": B1t,
                "B2t": B2t,
                "gidx": np.ascontiguousarray(gidx16[c]),
                "S": np.ascontiguousarray(S.reshape(128, TBQ * 128)),
            }
        )
    return in_maps, nbq.reshape(NG, NQ, GRP)


def _build(nbq):
    """Build + compile the SPMD Bass program. nbq: [NG, NQ, GRP] blocks."""
    QR = AGROWS // NQ
    NG = T // GRP
    TBQ = int(np.sum(nbq))
    nbg = nbq.reshape(NG, NQ * GRP).sum(axis=1)  # blocks per group
    nbt_max = int(nbg.max())
    gbase = np.concatenate([[0], np.cumsum(nbg)[:-1]])  # global block base per group
    f32 = mybir.dt.float32
    bf16 = mybir.dt.bfloat16
    i16 = mybir.dt.int16

    nc = bacc.Bacc(
        "TRN2",
        target_bir_lowering=False,
        debug=False,
        enable_asserts=False,
        num_devices=C,
        num_swdge_queues=NSWQ,
    )

    xT = nc.dram_tensor("xT", [DIN, NSHP], bf16, kind="ExternalInput")
    dinv_t = nc.dram_tensor("dinv_t", [128, T], f32, kind="ExternalInput")
    W1f = nc.dram_tensor("W1f", [DIN, HID], bf16, kind="ExternalInput")
    W2f = nc.dram_tensor("W2f", [HID, HID], bf16, kind="ExternalInput")
    B1t = nc.dram_tensor("B1t", [128, HID], f32, kind="ExternalInput")
    B2t = nc.dram_tensor("B2t", [128, HID], f32, kind="ExternalInput")
    gidx = nc.dram_tensor("gidx", [128, TBQ * 8], i16, kind="ExternalInput")
    Sdram = nc.dram_tensor("S", [128, TBQ * 128], bf16, kind="ExternalInput")
    out = nc.dram_tensor("out", [NSHP, HID], f32, kind="ExternalOutput")

    rg = [list(range(C))]

    with tile.TileContext(nc) as tc:
        with (
            tc.tile_pool(name="res", bufs=1) as res,
            tc.tile_pool(name="dram", bufs=1, space="DRAM") as dram,
            tc.tile_pool(name="xp", bufs=3) as xp,
            tc.tile_pool(name="hp", bufs=3) as hp,
            tc.tile_pool(name="gp", bufs=2) as gp,
            tc.tile_pool(name="ssp", bufs=2) as ssp,
            tc.tile_pool(name="ep", bufs=3) as ep,
            tc.tile_pool(name="pp", bufs=2, space="PSUM") as pp,
            tc.tile_pool(name="ap", bufs=2, space="PSUM") as ap_pool,
            tc.tile_pool(name="tp", bufs=2, space="PSUM") as tp,
        ):
            nc.gpsimd.load_library(library_config.mlp)
            gsem = [nc.alloc_semaphore(f"gsem{q}") for q in range(NSWQ)]

            # ---- resident SBUF tensors -------------------------------------
            gidx_sb = res.tile([128, TBQ * 8], i16)
            nc.sync.dma_start(out=gidx_sb[:], in_=gidx[:, :])
            dinv_sb = res.tile([128, T], f32)
            nc.sync.dma_start(out=dinv_sb[:], in_=dinv_t[:, :])
            B1_sb = res.tile([128, HID], f32)
            nc.sync.dma_start(out=B1_sb[:], in_=B1t[:, :])
            B2_sb = res.tile([128, HID], f32)
            nc.sync.dma_start(out=B2_sb[:], in_=B2t[:, :])
            W1_sb = res.tile([128, 2, HID], bf16)
            nc.sync.dma_start(
                out=W1_sb[:], in_=W1f[:, :].rearrange("(k p) h -> p k h", p=128)
            )
            W2_sb = res.tile([128, HID], bf16)
            nc.sync.dma_start(out=W2_sb[:], in_=W2f[:, :])
            ident_sb = res.tile([128, 128], bf16)
            make_identity(nc, ident_sb[:])
            h1T_sb = res.tile([128, NSHP], bf16)

            # ---- internal DRAM ---------------------------------------------
            h1loc = dram.tile([NSHP, HID], bf16)
            hg1 = dram.tile([AGROWS, HID], bf16, addr_space="Shared")
            h2loc = dram.tile([NSHP, HID], bf16)
            hg2 = dram.tile([AGROWS, HID], bf16, addr_space="Shared")

            # ---- phase 1: H1' = (X @ W1f) * dinv  --------------------------
            XCH = 8  # node tiles per xT load chunk
            for t0 in range(0, T, XCH):
                tn = min(XCH, T - t0)
                xa = xp.tile([128, 2, tn * 128], bf16, tag="xa")
                nc.sync.dma_start(
                    out=xa[:],
                    in_=xT[:, t0 * 128 : (t0 + tn) * 128].rearrange(
                        "(k p) n -> p k n", p=128
                    ),
                )
                for j in range(tn):
                    t = t0 + j
                    ps = pp.tile([128, HID], f32, tag="ps")
                    nc.tensor.matmul(
                        out=ps[:],
                        lhsT=xa[:, 0, j * 128 : (j + 1) * 128],
                        rhs=W1_sb[:, 0, :],
                        start=True,
                        stop=False,
                    )
                    nc.tensor.matmul(
                        out=ps[:],
                        lhsT=xa[:, 1, j * 128 : (j + 1) * 128],
                        rhs=W1_sb[:, 1, :],
                        start=False,
                        stop=True,
                    )
                    hrow = hp.tile([128, HID], bf16, tag="hrow")
                    nc.scalar.activation(
                        hrow[:],
                        ps[:],
                        mybir.ActivationFunctionType.Copy,
                        scale=dinv_sb[:, t : t + 1],
                    )
                    nc.sync.dma_start(
                        out=h1loc[t * 128 : (t + 1) * 128, :], in_=hrow[:]
                    )

            # ---- allgather layer-1 features --------------------------------
            nc.gpsimd.collective_compute(
                "AllGather",
                mybir.AluOpType.bypass,
                replica_groups=rg,
                ins=[h1loc[:, :].opt()],
                outs=[hg1[:, :].opt()],
            )

            def gather_group(hg, g, qoff):
                """One dma_gather per (group, round), pipelined via SWDGE queues."""
                gath = gp.tile([128, nbt_max, 128], bf16, tag="gath")
                brel = 0
                qi = qoff
                for q in range(NQ):
                    kq = int(nbq[g, q, :].sum())
                    if kq == 0:
                        continue
                    b0 = int(gbase[g]) + brel
                    qn = qi % NSWQ
                    qi += 1
                    if USE_PREP:
                        nc.gpsimd.dma_gather(
                            gath[:, brel : brel + kq, :],
                            hg[q * QR : (q + 1) * QR, :],
                            gidx_sb[:, b0 * 8 : (b0 + kq) * 8],
                            kq * 128,
                            kq * 128,
                            HID,
                            single_packet=False,
                            prepare_only=True,
                            sem=gsem[qn],
                            queue_num=qn,
                        )
                        nc.gpsimd.trigger_dma(count=None, queue_num=qn)
                    else:
                        nc.gpsimd.dma_gather(
                            gath[:, brel : brel + kq, :],
                            hg[q * QR : (q + 1) * QR, :],
                            gidx_sb[:, b0 * 8 : (b0 + kq) * 8],
                            kq * 128,
                            kq * 128,
                            HID,
                            single_packet=False,
                            queue_num=qn,
                        )
                    brel += kq
                return gath, qi

            def load_S(g):
                nbg_g = int(nbg[g])
                st = ssp.tile([128, nbt_max * 128], bf16, tag="S")
                nc.sync.dma_start(
                    out=st[:, : nbg_g * 128],
                    in_=Sdram[:, int(gbase[g]) * 128 : (int(gbase[g]) + nbg_g) * 128],
                )
                return st

            def tile_blocks(g, tw):
                """(block index within group) list for tile g*GRP+tw."""
                out_blocks = []
                brel = 0
                for q in range(NQ):
                    off = brel + int(nbq[g, q, :tw].sum())
                    for b in range(int(nbq[g, q, tw])):
                        out_blocks.append(off + b)
                    brel += int(nbq[g, q, :].sum())
                return out_blocks

            def aggregate(gath, Sg, g, tw):
                """Segment-sum one dst tile from the group gather tile."""
                blocks = tile_blocks(g, tw)
                acc = ap_pool.tile([128, HID], f32, tag="acc")
                for j, gb in enumerate(blocks):
                    nc.tensor.matmul(
                        out=acc[:],
                        lhsT=Sg[:, gb * 128 : (gb + 1) * 128],
                        rhs=gath[:, gb, :],
                        start=(j == 0),
                        stop=(j == len(blocks) - 1),
                    )
                return acc

            # ---- phase 2: aggregate layer 1, keep transposed in SBUF -------
            qi = 0
            for g in range(NG):
              Sg = load_S(g)
              gath1, qi = gather_group(hg1, g, qi)
              for tw in range(GRP):
                t = g * GRP + tw
                acc = aggregate(gath1, Sg, g, tw)
                e0 = ep.tile([128, HID], f32, tag="e0")
                nc.scalar.activation(
                    e0[:], acc[:], mybir.ActivationFunctionType.Copy,
                    scale=dinv_sb[:, t : t + 1],
                )
                e1 = ep.tile([128, HID], f32, tag="e1")
                nc.vector.tensor_add(e1[:], e0[:], B1_sb[:])
                r1 = ep.tile([128, HID], bf16, tag="r1")
                nc.scalar.activation(r1[:], e1[:], mybir.ActivationFunctionType.Relu)
                pt = tp.tile([128, 128], bf16, tag="pt")
                nc.tensor.transpose(out=pt[:], in_=r1[:], identity=ident_sb[:])
                nc.vector.tensor_copy(h1T_sb[:, t * 128 : (t + 1) * 128], pt[:])

            # ---- phase 3: H2' = (H1 @ W2f) * dinv --------------------------
            for t in range(T):
                ps2 = pp.tile([128, HID], f32, tag="ps2")
                nc.tensor.matmul(
                    out=ps2[:],
                    lhsT=h1T_sb[:, t * 128 : (t + 1) * 128],
                    rhs=W2_sb[:],
                    start=True,
                    stop=True,
                )
                hrow2 = hp.tile([128, HID], bf16, tag="hrow2")
                nc.scalar.activation(
                    hrow2[:],
                    ps2[:],
                    mybir.ActivationFunctionType.Copy,
                    scale=dinv_sb[:, t : t + 1],
                )
                nc.sync.dma_start(out=h2loc[t * 128 : (t + 1) * 128, :], in_=hrow2[:])

            # ---- allgather layer-2 features --------------------------------
            nc.gpsimd.collective_compute(
                "AllGather",
                mybir.AluOpType.bypass,
                replica_groups=rg,
                ins=[h2loc[:, :].opt()],
                outs=[hg2[:, :].opt()],
            )

            # ---- phase 4: aggregate layer 2 -> output ----------------------
            for g in range(NG):
              Sg2 = load_S(g)
              gath2, qi = gather_group(hg2, g, qi)
              for tw in range(GRP):
                t = g * GRP + tw
                acc2 = aggregate(gath2, Sg2, g, tw)
                e20 = ep.tile([128, HID], f32, tag="e20")
                nc.scalar.activation(
                    e20[:], acc2[:], mybir.ActivationFunctionType.Copy,
                    scale=dinv_sb[:, t : t + 1],
                )
                e2 = ep.tile([128, HID], f32, tag="e2")
                nc.vector.tensor_add(e2[:], e20[:], B2_sb[:])
                r2 = ep.tile([128, HID], f32, tag="r2")
                nc.scalar.activation(r2[:], e2[:], mybir.ActivationFunctionType.Relu)
                nc.sync.dma_start(out=out[t * 128 : (t + 1) * 128, :], in_=r2[:])

    nc.compile()
    return nc


def kernel(**inputs) -> np.ndarray:
    in_maps, nbq = _preprocess(inputs)
    nc = _build(nbq)
    nc.m = get_hw_module(nc.m)
    res = None
    last = None
    for _ in range(3):  # retry: a previously wedged device often clears
        try:
            res = run_bass_kernel_spmd(nc, in_maps, core_ids=list(range(C)))
            break
        except Exception as e:  # noqa: BLE001
            last = e
    if res is None:
        raise last
    shards = [np.asarray(res.results[c]["out"][:NSH]) for c in range(C)]
    return np.concatenate(shards, axis=0).astype(np.float32)


# revision 13
# speedup vs baseline: 1.0121x; 1.0121x over previous
"""GCN (2-layer, eval-mode BN, ReLU) on 8 Trainium2 NeuronCores.

Strategy: shard nodes across 8 cores (12500 each). Per layer:
  1. local matmul H = X @ (W * bn_scale), scaled per-node by dinv (bf16)
  2. AllGather bf16 node features -> full table in each core's DRAM
  3. per 128-node dst tile: dma_gather (custom Q7 ucode; int16 indices, so
     the table is addressed in 4 source-range rounds) pulls all incident
     edge messages into SBUF; gathers are issued prepare_only on 4 SWDGE
     queues so transfers overlap. The one-hot scatter matrices
     S[e,d] = (dloc[e]==d) are precomputed on host (pure 0/1; both norm
     factors are folded elsewhere), streamed from DRAM, and contracted
     against the gathered messages on the PE (the segment-sum).
  4. epilogue + ReLU; layer-1 result kept transposed in SBUF for layer 2.

All graph preprocessing (sorting edges by (dst tile, src range), padding to
128-edge blocks with a cross-core-uniform schedule, BN constant folding,
one-hot S blocks) happens on host.
"""

import math

import numpy as np
import ml_dtypes

import concourse.bass as bass
import concourse.mybir as mybir
import concourse.tile as tile
from concourse import bacc, library_config
from concourse.bass_utils import run_bass_kernel_spmd
from concourse.bass_interp import get_hw_module
from concourse.masks import make_identity

BF16 = ml_dtypes.bfloat16

N = 100000
E = 3200000
DIN = 256
HID = 128
C = 8
NSH = N // C            # 12500 nodes per core
T = math.ceil(NSH / 128)  # 98 dst tiles per core
NSHP = T * 128          # 12544 padded nodes per core
AGROWS = C * NSHP       # 100352 rows in the gathered table
NQ = 4                  # source-range rounds (int16 gather indices)
GRP = 2                 # dst tiles merged per gather call
BN_EPS = 1e-5
NSWQ = 4                # SWDGE queues for overlapped gathers

_PAD_DLOC = 200.0  # no local dst index matches 200 -> padding slot

USE_PREP = False   # prepare_only + trigger_dma pipelined gathers


def _fold_bn(W, b, gamma, beta, rm, rv):
    s = 1.0 / np.sqrt(np.asarray(rv, np.float64) + BN_EPS)
    A = np.asarray(gamma, np.float64) * s
    Bc = (np.asarray(b, np.float64) - np.asarray(rm, np.float64)) * A + np.asarray(
        beta, np.float64
    )
    Wf = np.asarray(W, np.float64) * A[None, :]
    return Wf.astype(BF16), Bc.astype(np.float32)


def _preprocess(inputs):
    QR = AGROWS // NQ
    assert QR <= 32767

    src = np.asarray(inputs["edge_index"][0], dtype=np.int64)
    dst = np.asarray(inputs["edge_index"][1], dtype=np.int64)
    loops = np.arange(N, dtype=np.int64)
    sa = np.concatenate([src, loops])
    da = np.concatenate([dst, loops])

    deg = np.bincount(da, minlength=N).astype(np.float64)
    dinv = (1.0 / np.sqrt(deg)).astype(np.float32)

    gv = (sa // NSH) * NSHP + (sa % NSH)  # row in the gathered table
    q_arr0 = gv // QR

    c_arr0 = da // NSH
    nloc0 = da - c_arr0 * NSH
    t_arr0 = nloc0 // 128

    NG = T // GRP  # tile groups (one merged gather call per (group, round))
    g_arr0 = t_arr0 // GRP
    tw0 = t_arr0 - g_arr0 * GRP
    key = ((c_arr0 * NG + g_arr0) * NQ + q_arr0) * GRP + tw0
    order = np.argsort(key, kind="stable")
    sa, da, gv, key = sa[order], da[order], gv[order], key[order]
    c_arr = c_arr0[order]
    nloc = nloc0[order]
    dl = (nloc - (nloc // 128) * 128).astype(np.float32)

    NSEC = NG * NQ * GRP
    cnt = np.bincount(key, minlength=C * NSEC)
    starts = np.concatenate([[0], np.cumsum(cnt)[:-1]])
    within = np.arange(len(da)) - starts[key]

    # cross-core-uniform block schedule per (group, round, tile)
    nbq = np.ceil(cnt.reshape(C, NSEC).max(axis=0) / 128).astype(np.int64)
    boffq = np.concatenate([[0], np.cumsum(nbq)[:-1]])
    TBQ = int(nbq.sum())

    tq = key - c_arr * NSEC  # section id per edge
    B_arr = boffq[tq] + within // 128
    p_arr = within % 128

    dloc = np.full((C, 128, TBQ), _PAD_DLOC, np.float32)
    dloc[c_arr, p_arr, B_arr] = dl

    # wrapped int16 index layout for dma_gather: per call (t,q), flat pos i
    # lives at [i%16 + 16k, boffq*8 + i//16] for k in 0..7 (replicated)
    lidx = (gv % QR).astype(np.int16)
    col_g = boffq[tq] * 8 + within // 16
    row_g = within % 16
    gidx16 = np.zeros((C, 128, TBQ * 8), np.int16)
    for k in range(8):
        gidx16[c_arr, row_g + 16 * k, col_g] = lidx

    W1f, B1 = _fold_bn(
        inputs["W1"], inputs["b1"], inputs["gamma1"], inputs["beta1"],
        inputs["rm1"], inputs["rv1"],
    )
    W2f, B2 = _fold_bn(
        inputs["W2"], inputs["b2"], inputs["gamma2"], inputs["beta2"],
        inputs["rm2"], inputs["rv2"],
    )
    B1t = np.ascontiguousarray(np.broadcast_to(B1, (128, HID)))
    B2t = np.ascontiguousarray(np.broadcast_to(B2, (128, HID)))

    x = np.asarray(inputs["x"], np.float32)
    d_iota = np.arange(128, dtype=np.float32)
    in_maps = []
    for c in range(C):
        xs = x[c * NSH : (c + 1) * NSH]  # [NSH, DIN]
        xT = np.zeros((DIN, NSHP), BF16)
        xT[:, :NSH] = xs.T.astype(BF16)
        dvflat = np.zeros(NSHP, np.float32)
        dvflat[:NSH] = dinv[c * NSH : (c + 1) * NSH]
        dinv_t = np.ascontiguousarray(dvflat.reshape(T, 128).T)  # [128, T]
        # host-built one-hot scatter blocks: S[p, b, d] = (dloc[c,p,b] == d)
        S = (dloc[c][:, :, None] == d_iota[None, None, :]).astype(BF16)
        in_maps.append(
            {
                "xT": xT,
                "dinv_t": dinv_t,
                "W1f": W1f,
                "W2f": W2f,
                "B1t": B1t,
                "B2t": B2t,
                "gidx": np.ascontiguousarray(gidx16[c]),
                "S": np.ascontiguousarray(S.reshape(128, TBQ * 128)),
            }
        )
    return in_maps, nbq.reshape(NG, NQ, GRP)


def _build(nbq):
    """Build + compile the SPMD Bass program. nbq: [NG, NQ, GRP] blocks."""
    QR = AGROWS // NQ
    NG = T // GRP
    TBQ = int(np.sum(nbq))
    nbg = nbq.reshape(NG, NQ * GRP).sum(axis=1)  # blocks per group
    nbt_max = int(nbg.max())
    gbase = np.concatenate([[0], np.cumsum(nbg)[:-1]])  # global block base per group
    f32 = mybir.dt.float32
    bf16 = mybir.dt.bfloat16
    i16 = mybir.dt.int16

    nc = bacc.Bacc(
        "TRN2",
        target_bir_lowering=False,
        debug=False,
        enable_asserts=False,
        num_devices=C,
        num_swdge_queues=NSWQ,
    )

    xT = nc.dram_tensor("xT", [DIN, NSHP], bf16, kind="ExternalInput")
    dinv_t = nc.dram_tensor("dinv_t", [128, T], f32, kind="ExternalInput")
    W1f = nc.dram_tensor("W1f", [DIN, HID], bf16, kind="ExternalInput")
    W2f = nc.dram_tensor("W2f", [HID, HID], bf16, kind="ExternalInput")
    B1t = nc.dram_tensor("B1t", [128, HID], f32, kind="ExternalInput")
    B2t = nc.dram_tensor("B2t", [128, HID], f32, kind="ExternalInput")
    gidx = nc.dram_tensor("gidx", [128, TBQ * 8], i16, kind="ExternalInput")
    Sdram = nc.dram_tensor("S", [128, TBQ * 128], bf16, kind="ExternalInput")
    out = nc.dram_tensor("out", [NSHP, HID], f32, kind="ExternalOutput")

    rg = [list(range(C))]

    with tile.TileContext(nc) as tc:
        with (
            tc.tile_pool(name="res", bufs=1) as res,
            tc.tile_pool(name="dram", bufs=1, space="DRAM") as dram,
            tc.tile_pool(name="xp", bufs=2) as xp,
            tc.tile_pool(name="hp", bufs=3) as hp,
            tc.tile_pool(name="gp", bufs=3) as gp,
            tc.tile_pool(name="ssp", bufs=2) as ssp,
            tc.tile_pool(name="ep", bufs=3) as ep,
            tc.tile_pool(name="pp", bufs=2, space="PSUM") as pp,
            tc.tile_pool(name="ap", bufs=2, space="PSUM") as ap_pool,
            tc.tile_pool(name="tp", bufs=2, space="PSUM") as tp,
        ):
            nc.gpsimd.load_library(library_config.mlp)
            gsem = [nc.alloc_semaphore(f"gsem{q}") for q in range(NSWQ)]

            # ---- resident SBUF tensors -------------------------------------
            gidx_sb = res.tile([128, TBQ * 8], i16)
            nc.sync.dma_start(out=gidx_sb[:], in_=gidx[:, :])
            dinv_sb = res.tile([128, T], f32)
            nc.sync.dma_start(out=dinv_sb[:], in_=dinv_t[:, :])
            B1_sb = res.tile([128, HID], f32)
            nc.sync.dma_start(out=B1_sb[:], in_=B1t[:, :])
            B2_sb = res.tile([128, HID], f32)
            nc.sync.dma_start(out=B2_sb[:], in_=B2t[:, :])
            W1_sb = res.tile([128, 2, HID], bf16)
            nc.sync.dma_start(
                out=W1_sb[:], in_=W1f[:, :].rearrange("(k p) h -> p k h", p=128)
            )
            W2_sb = res.tile([128, HID], bf16)
            nc.sync.dma_start(out=W2_sb[:], in_=W2f[:, :])
            ident_sb = res.tile([128, 128], bf16)
            make_identity(nc, ident_sb[:])
            h1T_sb = res.tile([128, NSHP], bf16)

            # ---- internal DRAM ---------------------------------------------
            h1loc = dram.tile([NSHP, HID], bf16)
            hg1 = dram.tile([AGROWS, HID], bf16, addr_space="Shared")
            h2loc = dram.tile([NSHP, HID], bf16)
            hg2 = dram.tile([AGROWS, HID], bf16, addr_space="Shared")

            # ---- phase 1: H1' = (X @ W1f) * dinv  --------------------------
            XCH = 8  # node tiles per xT load chunk
            for t0 in range(0, T, XCH):
                tn = min(XCH, T - t0)
                xa = xp.tile([128, 2, tn * 128], bf16, tag="xa")
                nc.sync.dma_start(
                    out=xa[:],
                    in_=xT[:, t0 * 128 : (t0 + tn) * 128].rearrange(
                        "(k p) n -> p k n", p=128
                    ),
                )
                for j in range(tn):
                    t = t0 + j
                    ps = pp.tile([128, HID], f32, tag="ps")
                    nc.tensor.matmul(
                        out=ps[:],
                        lhsT=xa[:, 0, j * 128 : (j + 1) * 128],
                        rhs=W1_sb[:, 0, :],
                        start=True,
                        stop=False,
                    )
                    nc.tensor.matmul(
                        out=ps[:],
                        lhsT=xa[:, 1, j * 128 : (j + 1) * 128],
                        rhs=W1_sb[:, 1, :],
                        start=False,
                        stop=True,
                    )
                    hrow = hp.tile([128, HID], bf16, tag="hrow")
                    nc.scalar.activation(
                        hrow[:],
                        ps[:],
                        mybir.ActivationFunctionType.Copy,
                        scale=dinv_sb[:, t : t + 1],
                    )
                    nc.sync.dma_start(
                        out=h1loc[t * 128 : (t + 1) * 128, :], in_=hrow[:]
                    )

            # ---- allgather layer-1 features --------------------------------
            nc.gpsimd.collective_compute(
                "AllGather",
                mybir.AluOpType.bypass,
                replica_groups=rg,
                ins=[h1loc[:, :].opt()],
                outs=[hg1[:, :].opt()],
            )

            def gather_group(hg, g, qoff):
                """One dma_gather per (group, round), pipelined via SWDGE queues."""
                gath = gp.tile([128, nbt_max, 128], bf16, tag="gath")
                brel = 0
                qi = qoff
                for q in range(NQ):
                    kq = int(nbq[g, q, :].sum())
                    if kq == 0:
                        continue
                    b0 = int(gbase[g]) + brel
                    qn = qi % NSWQ
                    qi += 1
                    if USE_PREP:
                        nc.gpsimd.dma_gather(
                            gath[:, brel : brel + kq, :],
                            hg[q * QR : (q + 1) * QR, :],
                            gidx_sb[:, b0 * 8 : (b0 + kq) * 8],
                            kq * 128,
                            kq * 128,
                            HID,
                            single_packet=False,
                            prepare_only=True,
                            sem=gsem[qn],
                            queue_num=qn,
                        )
                        nc.gpsimd.trigger_dma(count=None, queue_num=qn)
                    else:
                        nc.gpsimd.dma_gather(
                            gath[:, brel : brel + kq, :],
                            hg[q * QR : (q + 1) * QR, :],
                            gidx_sb[:, b0 * 8 : (b0 + kq) * 8],
                            kq * 128,
                            kq * 128,
                            HID,
                            single_packet=False,
                            queue_num=qn,
                        )
                    brel += kq
                return gath, qi

            def load_S(g):
                nbg_g = int(nbg[g])
                st = ssp.tile([128, nbt_max * 128], bf16, tag="S")
                nc.sync.dma_start(
                    out=st[:, : nbg_g * 128],
                    in_=Sdram[:, int(gbase[g]) * 128 : (int(gbase[g]) + nbg_g) * 128],
                )
                return st

            def tile_blocks(g, tw):
                """(block index within group) list for tile g*GRP+tw."""
                out_blocks = []
                brel = 0
                for q in range(NQ):
                    off = brel + int(nbq[g, q, :tw].sum())
                    for b in range(int(nbq[g, q, tw])):
                        out_blocks.append(off + b)
                    brel += int(nbq[g, q, :].sum())
                return out_blocks

            def aggregate(gath, Sg, g, tw):
                """Segment-sum one dst tile from the group gather tile."""
                blocks = tile_blocks(g, tw)
                acc = ap_pool.tile([128, HID], f32, tag="acc")
                for j, gb in enumerate(blocks):
                    nc.tensor.matmul(
                        out=acc[:],
                        lhsT=Sg[:, gb * 128 : (gb + 1) * 128],
                        rhs=gath[:, gb, :],
                        start=(j == 0),
                        stop=(j == len(blocks) - 1),
                    )
                return acc

            # ---- phase 2: aggregate layer 1, keep transposed in SBUF -------
            qi = 0
            for g in range(NG):
              Sg = load_S(g)
              gath1, qi = gather_group(hg1, g, qi)
              for tw in range(GRP):
                t = g * GRP + tw
                acc = aggregate(gath1, Sg, g, tw)
                e0 = ep.tile([128, HID], f32, tag="e0")
                nc.scalar.activation(
                    e0[:], acc[:], mybir.ActivationFunctionType.Copy,
                    scale=dinv_sb[:, t : t + 1],
                )
                e1 = ep.tile([128, HID], f32, tag="e1")
                nc.vector.tensor_add(e1[:], e0[:], B1_sb[:])
                r1 = ep.tile([128, HID], bf16, tag="r1")
                nc.scalar.activation(r1[:], e1[:], mybir.ActivationFunctionType.Relu)
                pt = tp.tile([128, 128], bf16, tag="pt")
                nc.tensor.transpose(out=pt[:], in_=r1[:], identity=ident_sb[:])
                nc.vector.tensor_copy(h1T_sb[:, t * 128 : (t + 1) * 128], pt[:])

            # ---- phase 3: H2' = (H1 @ W2f) * dinv --------------------------
            for t in range(T):
                ps2 = pp.tile([128, HID], f32, tag="ps2")
                nc.tensor.matmul(
                    out=ps2[:],
                    lhsT=h1T_sb[:, t * 128 : (t + 1) * 128],
                    rhs=W2_sb[:],
                    start=True,
                    stop=True,
                )
                hrow2 = hp.tile([128, HID], bf16, tag="hrow2")
                nc.scalar.activation(
                    hrow2[:],
                    ps2[:],
                    mybir.ActivationFunctionType.Copy,
                    scale=dinv_sb[:, t : t + 1],
                )
                nc.sync.dma_start(out=h2loc[t * 128 : (t + 1) * 128, :], in_=hrow2[:])

            # ---- allgather layer-2 features --------------------------------
            nc.gpsimd.collective_compute(
                "AllGather",
                mybir.AluOpType.bypass,
                replica_groups=rg,
                ins=[h2loc[:, :].opt()],
                outs=[hg2[:, :].opt()],
            )

            # ---- phase 4: aggregate layer 2 -> output ----------------------
            for g in range(NG):
              Sg2 = load_S(g)
              gath2, qi = gather_group(hg2, g, qi)
              for tw in range(GRP):
                t = g * GRP + tw
                acc2 = aggregate(gath2, Sg2, g, tw)
                e20 = ep.tile([128, HID], f32, tag="e20")
                nc.scalar.activation(
                    e20[:], acc2[:], mybir.ActivationFunctionType.Copy,
                    scale=dinv_sb[:, t : t + 1],
                )
                e2 = ep.tile([128, HID], f32, tag="e2")
                nc.vector.tensor_add(e2[:], e20[:], B2_sb[:])
                r2 = ep.tile([128, HID], f32, tag="r2")
                nc.scalar.activation(r2[:], e2[:], mybir.ActivationFunctionType.Relu)
                nc.sync.dma_start(out=out[t * 128 : (t + 1) * 128, :], in_=r2[:])

    nc.compile()
    return nc


def kernel(**inputs) -> np.ndarray:
    in_maps, nbq = _preprocess(inputs)
    nc = _build(nbq)
    nc.m = get_hw_module(nc.m)
    res = None
    last = None
    for _ in range(3):  # retry: a previously wedged device often clears
        try:
            res = run_bass_kernel_spmd(nc, in_maps, core_ids=list(range(C)))
            break
        except Exception as e:  # noqa: BLE001
            last = e
    if res is None:
        raise last
    shards = [np.asarray(res.results[c]["out"][:NSH]) for c in range(C)]
    return np.concatenate(shards, axis=0).astype(np.float32)
